# revision 7
# baseline (speedup 1.0000x reference)
"""Trainium2 Bass kernel for the AggregPolicy GNN message-passing model.

Math: the reference network is, per batch row x (18 features):
    s0 = E @ x_feats + d          (state s = [h_j[0..6] (7*4), h_m (4)] = 32 dims)
    s  = tanh(M @ s + c)          x 7   (chain-graph message passing folded into
                                         one dense 32x32 matrix M)
    out = F @ s + bact            (7 outputs)
The first iteration is fused with the init affine: s1 = tanh(G @ x + g) with
G = M @ E, g = M @ d + c.  All matrices are precomputed on the host from the
tiny model weights.

Layout on chip (per NeuronCore, pure data parallel over 8 cores):
  - 4 batch "chunks" x 32 state dims are stacked on the 128 SBUF partitions
    (block-diagonal G/M/F), batch runs along the free dimension.
  - batch rows are interleaved mod 4 across chunks, so each SBUF partition of
    an input tile holds 4 *consecutive* DRAM rows (288 B contiguous reads) and
    each partition of an output tile holds 4 consecutive rows of y (112 B
    contiguous writes).
  - Per 128-column tile: PE transposes x [128,72] -> [72,128] (features to
    partitions), then one matmul per iteration per 512-col slice; ScalarE
    applies tanh+bias straight PSUM->SBUF; final per-tile matmul uses the
    state tile itself as the stationary operand to emit batch-major outputs.
"""

import os

os.environ.setdefault("MYCRO_LOCAL_CACHE", "1")

from contextlib import ExitStack

import numpy as np

import concourse.bacc as bacc
import concourse.tile as tile
from concourse import mybir
from concourse.bass_utils import run_bass_kernel_spmd

F32 = mybir.dt.float32

N_CORES = 8
B_TOTAL = 2_000_000
R = B_TOTAL // N_CORES  # 250000 rows per core
NF = 18  # input features
NS = 32  # state dims
NO = 7  # outputs
CH = 4  # batch chunks stacked on partitions (4*32 = 128)
T_FULL = 128  # batch columns per tile (rows per tile = 4*T_FULL = 512)
BB_TILES = 8  # tiles per big-batch (ACT op free dim = 128*BB_TILES)

FULL_TILES = R // (CH * T_FULL)  # 488
TAIL_ROWS = R - FULL_TILES * CH * T_FULL  # 144
TAIL_T = TAIL_ROWS // CH  # 36
N_BB = FULL_TILES // BB_TILES  # 61
assert N_BB * BB_TILES == FULL_TILES
assert TAIL_T * CH == TAIL_ROWS


def build_host_constants(Wj, bj, Wm, bm, Wih_j, Whh_j, bih_j, bhh_j,
                         Wih_m, Whh_m, bih_m, bhh_m, Wact, bact):
    """Fold the model into (G_lhsT [72,128], M_lhsT [128,128], F_rhs [128,28],
    bias [128,3]) in the block-diagonal on-chip layouts."""
    H = 4
    M = np.zeros((NS, NS), np.float32)
    c = np.zeros((NS,), np.float32)
    for n in range(7):
        r = slice(4 * n, 4 * n + 4)
        if n == 0:
            M[r, 28:32] += Wih_j  # prev neighbor of node 0 is h_m
        else:
            M[r, 4 * (n - 1):4 * n] += Wih_j
        if n < 6:
            M[r, 4 * (n + 1):4 * (n + 2)] += Wih_j
        M[r, 4 * n:4 * n + 4] += Whh_j
        c[r] = bih_j + bhh_j
    M[28:32, 0:4] = Wih_m
    M[28:32, 28:32] = Whh_m
    c[28:32] = bih_m + bhh_m

    E = np.zeros((NS, NF), np.float32)
    d = np.zeros((NS,), np.float32)
    for n in range(7):
        for h in range(H):
            E[4 * n + h, 4 + n] = Wj[h, 0]
            E[4 * n + h, 11 + n] = Wj[h, 1]
        d[4 * n:4 * n + 4] = bj
    E[28:32, 0:4] = Wm
    d[28:32] = bm

    F = np.zeros((NO, NS), np.float32)
    for n in range(7):
        F[n, 4 * n:4 * n + 4] = Wact[0]

    G = (M @ E).astype(np.float32)
    g = (M @ d + c).astype(np.float32)

    # Block-diagonal device layouts.
    # G matmul: out[32c+o, col] = sum_f G[o,f] * xT[18c+f, col]
    G_lhsT = np.zeros((CH * NF, 128), np.float32)
    for cc in range(CH):
        G_lhsT[NF * cc:NF * (cc + 1), NS * cc:NS * (cc + 1)] = G.T
    # M matmul: out[32c+o, col] = sum_k M[o,k] * s[32c+k, col]
    M_lhsT = np.zeros((128, 128), np.float32)
    for cc in range(CH):
        M_lhsT[NS * cc:NS * (cc + 1), NS * cc:NS * (cc + 1)] = M.T
    # Final: out[t, 7c+j] = sum_{o} s7[32c+o, t] * F_rhs[32c+o, 7c+j]
    F_rhs = np.zeros((128, CH * NO), np.float32)
    for cc in range(CH):
        F_rhs[NS * cc:NS * (cc + 1), NO * cc:NO * (cc + 1)] = F.T
    # Per-partition bias vectors: col 0 = g (first iter), col 1 = c, col 2 = bact
    bias = np.zeros((128, 3), np.float32)
    bias[:, 0] = np.tile(g, CH)
    bias[:, 1] = np.tile(c, CH)
    bias[:, 2] = float(bact[0])
    identity = np.eye(128, dtype=np.float32)
    return G_lhsT, M_lhsT, F_rhs, bias, identity


def build_tile_kernel(ctx, tc, x, y, gw, mw, fw, idm, bvec, rows, nrep=1):
    """Emit the Tile program. x:[rows,18], y:[rows,7] DRAM APs; consts in DRAM.

    nrep > 1 wraps the whole body in a hardware For_i loop that recomputes the
    same outputs nrep times — used only for wall-clock benchmarking."""
    nc = tc.nc
    full_tiles = rows // (CH * T_FULL)
    tail_rows = rows - full_tiles * CH * T_FULL
    tail_t = tail_rows // CH
    assert tail_t * CH == tail_rows
    n_bb = (full_tiles + BB_TILES - 1) // BB_TILES

    consts = ctx.enter_context(tc.tile_pool(name="consts", bufs=1))
    xsb_pool = ctx.enter_context(tc.tile_pool(name="xsb", bufs=2))
    xf_pool = ctx.enter_context(tc.tile_pool(name="xfeat", bufs=2))
    s_pool = ctx.enter_context(tc.tile_pool(name="spool", bufs=2))
    osb_pool = ctx.enter_context(tc.tile_pool(name="osb", bufs=2))
    xt_pool = ctx.enter_context(tc.tile_pool(name="xt", bufs=2, space="PSUM"))
    pre_pool = ctx.enter_context(tc.tile_pool(name="pre", bufs=2, space="PSUM"))
    ops_pool = ctx.enter_context(tc.tile_pool(name="ops", bufs=2, space="PSUM"))

    g_sb = consts.tile([CH * NF, 128], F32, name="g_sb")
    nc.sync.dma_start(g_sb[:], gw)
    m_sb = consts.tile([128, 128], F32, name="m_sb")
    nc.sync.dma_start(m_sb[:], mw)
    f_sb = consts.tile([128, CH * NO], F32, name="f_sb")
    nc.sync.dma_start(f_sb[:], fw)
    id_sb = consts.tile([128, 128], F32, name="id_sb")
    nc.sync.dma_start(id_sb[:], idm)
    b_sb = consts.tile([128, 3], F32, name="b_sb")
    nc.sync.dma_start(b_sb[:], bvec)

    TANH = mybir.ActivationFunctionType.Tanh

    def do_bb(row0, ntile, t_last):
        """One big-batch: `ntile` tiles; tiles 0..ntile-2 are full (T=128),
        the last has T = t_last columns (4*t_last rows)."""
        widths = [T_FULL] * (ntile - 1) + [t_last]
        ncols = sum(widths)
        nrows = CH * ncols
        dense = all(w == T_FULL for w in widths)

        # ---- load x ----
        x_sb = xsb_pool.tile([128, NF * CH * ntile], F32, name="x_sb", tag="x_sb")
        if dense:
            src = x[row0:row0 + nrows, :].rearrange(
                "(k t c) f -> t k c f", k=ntile, t=T_FULL, c=CH)
            dst = x_sb[:, 0:NF * CH * ntile].rearrange(
                "t (k c f) -> t k c f", k=ntile, c=CH, f=NF)
            nc.sync.dma_start(dst, src)
        else:
            for ti, w in enumerate(widths):
                r0 = row0 + CH * sum(widths[:ti])
                src = x[r0:r0 + CH * w, :].rearrange("(t c) f -> t c f", t=w, c=CH)
                dst = x_sb[0:w, NF * CH * ti:NF * CH * (ti + 1)].rearrange(
                    "t (c f) -> t c f", c=CH, f=NF)
                nc.sync.dma_start(dst, src)

        # ---- transpose x tiles: [T, 72] -> [72, T] (features to partitions) ----
        xfeat = xf_pool.tile([CH * NF, T_FULL * ntile], F32, name="xfeat", tag="xfeat")
        for xg0 in range(0, ntile, 4):
            gcnt = min(4, ntile - xg0)
            xt = xt_pool.tile([CH * NF, 512], F32, name="xt", tag="xt")
            wsum = 0
            for i in range(gcnt):
                ti = xg0 + i
                w = widths[ti]
                nc.tensor.transpose(
                    out=xt[0:CH * NF, 128 * i:128 * i + w],
                    in_=x_sb[0:w, NF * CH * ti:NF * CH * (ti + 1)],
                    identity=id_sb[0:w, 0:w],
                )
                wsum += w
            if dense:
                nc.vector.tensor_copy(
                    xfeat[0:CH * NF, 128 * xg0:128 * xg0 + 128 * gcnt],
                    xt[0:CH * NF, 0:128 * gcnt])
            else:
                for i in range(gcnt):
                    ti = xg0 + i
                    w = widths[ti]
                    nc.vector.tensor_copy(
                        xfeat[0:CH * NF, 128 * ti:128 * ti + w],
                        xt[0:CH * NF, 128 * i:128 * i + w])

        # ---- iteration 1 (init fused): pre = Gbig @ xfeat ; s = tanh(pre + g) ----
        def mm_sliced(dst_tile, lhsT, rhs_tile, rhs_parts, width):
            for q0 in range(0, width, 512):
                w = min(512, width - q0)
                nc.tensor.matmul(
                    dst_tile[:, q0:q0 + w],
                    lhsT=lhsT,
                    rhs=rhs_tile[0:rhs_parts, q0:q0 + w],
                    start=True, stop=True)

        used_cols = T_FULL * (ntile - 1) + t_last  # valid columns in xfeat/s
        pre = pre_pool.tile([128, T_FULL * ntile], F32, name="pre", tag="pre")
        mm_sliced(pre, g_sb[:], xfeat, CH * NF, used_cols)
        s = s_pool.tile([128, T_FULL * ntile], F32, name="s", tag="s")
        nc.scalar.activation(s[:, 0:used_cols], pre[:, 0:used_cols], TANH,
                             bias=b_sb[:, 0:1], scale=1.0)

        # ---- iterations 2..7 ----
        for _ in range(6):
            pre2 = pre_pool.tile([128, T_FULL * ntile], F32, name="pre2", tag="pre")
            mm_sliced(pre2, m_sb[:], s, 128, used_cols)
            s2 = s_pool.tile([128, T_FULL * ntile], F32, name="s2", tag="s")
            nc.scalar.activation(s2[:, 0:used_cols], pre2[:, 0:used_cols], TANH,
                                 bias=b_sb[:, 1:2], scale=1.0)
            s = s2

        # ---- final: per tile, out[t, 28] = s7_tile.T @ F_rhs ----
        outps = ops_pool.tile([128, CH * NO * ntile], F32, name="outps", tag="outps")
        for ti, w in enumerate(widths):
            nc.tensor.matmul(
                outps[0:w, CH * NO * ti:CH * NO * (ti + 1)],
                lhsT=s[:, 128 * ti:128 * ti + w],
                rhs=f_sb[:],
                start=True, stop=True)
        out_sb = osb_pool.tile([128, CH * NO * ntile], F32, name="out_sb", tag="out_sb")
        if dense:
            nc.vector.tensor_scalar_add(out_sb[:], outps[:], b_sb[:, 2:3])
        else:
            for ti, w in enumerate(widths):
                nc.vector.tensor_scalar_add(
                    out_sb[0:w, CH * NO * ti:CH * NO * (ti + 1)],
                    outps[0:w, CH * NO * ti:CH * NO * (ti + 1)],
                    b_sb[0:w, 2:3])

        # ---- store y ----
        if dense:
            dsty = y[row0:row0 + nrows, :].rearrange(
                "(k t c) j -> t k c j", k=ntile, t=T_FULL, c=CH)
            srcy = out_sb[:, 0:CH * NO * ntile].rearrange(
                "t (k c j) -> t k c j", k=ntile, c=CH, j=NO)
            nc.sync.dma_start(dsty, srcy)
        else:
            for ti, w in enumerate(widths):
                r0 = row0 + CH * sum(widths[:ti])
                dsty = y[r0:r0 + CH * w, :].rearrange("(t c) j -> t c j", t=w, c=CH)
                srcy = out_sb[0:w, CH * NO * ti:CH * NO * (ti + 1)].rearrange(
                    "t (c j) -> t c j", c=CH, j=NO)
                nc.sync.dma_start(dsty, srcy)

    def emit_all():
        n_full_bb = full_tiles // BB_TILES
        for bb in range(n_full_bb):
            do_bb(bb * BB_TILES * CH * T_FULL, BB_TILES, T_FULL)
        leftover_tiles = full_tiles - n_full_bb * BB_TILES
        row0 = n_full_bb * BB_TILES * CH * T_FULL
        if tail_t > 0:
            do_bb(row0, leftover_tiles + 1, tail_t)
        elif leftover_tiles:
            do_bb(row0, leftover_tiles, T_FULL)

    if nrep == 1:
        emit_all()
    else:
        with tc.For_i(0, nrep, 1):
            emit_all()


_CACHED = {}
PROFILE = False  # set True (e.g. from test.py) to capture an NTFF trace
LAST_RESULTS = None  # BassKernelResults of the most recent kernel() call


def _build_program(rows, nrep=1):
    nc = bacc.Bacc("TRN2", target_bir_lowering=False, debug=False,
                   num_devices=N_CORES)
    x = nc.dram_tensor("x", [rows, NF], F32, kind="ExternalInput").ap()
    y = nc.dram_tensor("y", [rows, NO], F32, kind="ExternalOutput").ap()
    gw = nc.dram_tensor("gw", [CH * NF, 128], F32, kind="ExternalInput").ap()
    mw = nc.dram_tensor("mw", [128, 128], F32, kind="ExternalInput").ap()
    fw = nc.dram_tensor("fw", [128, CH * NO], F32, kind="ExternalInput").ap()
    idm = nc.dram_tensor("idm", [128, 128], F32, kind="ExternalInput").ap()
    bvec = nc.dram_tensor("bvec", [128, 3], F32, kind="ExternalInput").ap()
    with tile.TileContext(nc) as tc, ExitStack() as ctx:
        build_tile_kernel(ctx, tc, x, y, gw, mw, fw, idm, bvec, rows, nrep=nrep)
    nc.compile()
    return nc


def kernel(x, Wj, bj, Wm, bm, Wih_j, Whh_j, bih_j, bhh_j,
           Wih_m, Whh_m, bih_m, bhh_m, Wact, bact):
    x = np.ascontiguousarray(np.asarray(x, dtype=np.float32))
    assert x.shape == (B_TOTAL, NF), x.shape
    G_lhsT, M_lhsT, F_rhs, bias, identity = build_host_constants(
        np.asarray(Wj, np.float32), np.asarray(bj, np.float32),
        np.asarray(Wm, np.float32), np.asarray(bm, np.float32),
        np.asarray(Wih_j, np.float32), np.asarray(Whh_j, np.float32),
        np.asarray(bih_j, np.float32), np.asarray(bhh_j, np.float32),
        np.asarray(Wih_m, np.float32), np.asarray(Whh_m, np.float32),
        np.asarray(bih_m, np.float32), np.asarray(bhh_m, np.float32),
        np.asarray(Wact, np.float32), np.asarray(bact, np.float32))

    if "nc" not in _CACHED:
        _CACHED["nc"] = _build_program(R)
    nc = _CACHED["nc"]

    in_maps = []
    for i in range(N_CORES):
        in_maps.append({
            "x": x[i * R:(i + 1) * R],
            "gw": G_lhsT, "mw": M_lhsT, "fw": F_rhs,
            "idm": identity, "bvec": bias,
        })
    res = run_bass_kernel_spmd(nc, in_maps, list(range(N_CORES)), trace=PROFILE)
    global LAST_RESULTS
    LAST_RESULTS = res
    out = np.concatenate([res.results[i]["y"] for i in range(N_CORES)], axis=0)
    return out


# revision 16
# speedup vs baseline: 1.6635x; 1.6635x over previous
"""Trainium2 Bass kernel for the AggregPolicy GNN message-passing model.

Math: the reference network is, per batch row x (18 features):
    s0 = E @ x_feats + d          (state s = [h_j[0..6] (7*4), h_m (4)] = 32 dims)
    s  = tanh(M @ s + c)          x 7   (chain-graph message passing folded into
                                         one dense 32x32 matrix M)
    out = F @ s + bact            (7 outputs)
The first iteration is fused with the init affine: s1 = tanh(G @ x + g) with
G = M @ E, g = M @ d + c.  All matrices are precomputed on the host from the
tiny model weights.

Layout on chip (per NeuronCore, pure data parallel over 8 cores):
  - 4 batch "chunks" x 32 state dims are stacked on the 128 SBUF partitions
    (block-diagonal G/M/F), batch runs along the free dimension.
  - batch rows are interleaved mod 4 across chunks, so each SBUF partition of
    an input tile holds 4 *consecutive* DRAM rows (288 B contiguous reads) and
    each partition of an output tile holds 4 consecutive rows of y (112 B
    contiguous writes).
  - Per 128-column tile: PE transposes x [128,72] -> [72,128] (features to
    partitions), then one matmul per iteration per 512-col slice; ScalarE
    applies tanh+bias straight PSUM->SBUF; final per-tile matmul uses the
    state tile itself as the stationary operand to emit batch-major outputs.
"""

import os

os.environ.setdefault("MYCRO_LOCAL_CACHE", "1")

from contextlib import ExitStack

import numpy as np

import concourse.bacc as bacc
import concourse.tile as tile
from concourse import mybir
from concourse.bass_utils import run_bass_kernel_spmd

F32 = mybir.dt.float32

N_CORES = 8
B_TOTAL = 2_000_000
R = B_TOTAL // N_CORES  # 250000 rows per core
NF = 18  # input features
NS = 32  # state dims
NO = 7  # outputs
CH = 4  # batch chunks stacked on partitions (4*32 = 128)
T_FULL = 128  # batch columns per tile (rows per tile = 4*T_FULL = 512)
BB_TILES = 12  # tiles per big-batch (ACT op free dim = 128*BB_TILES)

FULL_TILES = R // (CH * T_FULL)  # 488
TAIL_ROWS = R - FULL_TILES * CH * T_FULL  # 144
TAIL_T = TAIL_ROWS // CH  # 36
assert TAIL_T * CH == TAIL_ROWS


def build_host_constants(Wj, bj, Wm, bm, Wih_j, Whh_j, bih_j, bhh_j,
                         Wih_m, Whh_m, bih_m, bhh_m, Wact, bact):
    """Fold the model into (G_lhsT [72,128], M_lhsT [128,128], F_rhs [128,28],
    bias [128,3]) in the block-diagonal on-chip layouts."""
    H = 4
    M = np.zeros((NS, NS), np.float32)
    c = np.zeros((NS,), np.float32)
    for n in range(7):
        r = slice(4 * n, 4 * n + 4)
        if n == 0:
            M[r, 28:32] += Wih_j  # prev neighbor of node 0 is h_m
        else:
            M[r, 4 * (n - 1):4 * n] += Wih_j
        if n < 6:
            M[r, 4 * (n + 1):4 * (n + 2)] += Wih_j
        M[r, 4 * n:4 * n + 4] += Whh_j
        c[r] = bih_j + bhh_j
    M[28:32, 0:4] = Wih_m
    M[28:32, 28:32] = Whh_m
    c[28:32] = bih_m + bhh_m

    E = np.zeros((NS, NF), np.float32)
    d = np.zeros((NS,), np.float32)
    for n in range(7):
        for h in range(H):
            E[4 * n + h, 4 + n] = Wj[h, 0]
            E[4 * n + h, 11 + n] = Wj[h, 1]
        d[4 * n:4 * n + 4] = bj
    E[28:32, 0:4] = Wm
    d[28:32] = bm

    F = np.zeros((NO, NS), np.float32)
    for n in range(7):
        F[n, 4 * n:4 * n + 4] = Wact[0]

    G = (M @ E).astype(np.float32)
    g = (M @ d + c).astype(np.float32)

    # Block-diagonal device layouts.
    # G matmul: out[32c+o, col] = sum_f G[o,f] * xT[18c+f, col]
    G_lhsT = np.zeros((CH * NF, 128), np.float32)
    for cc in range(CH):
        G_lhsT[NF * cc:NF * (cc + 1), NS * cc:NS * (cc + 1)] = G.T
    # M matmul: out[32c+o, col] = sum_k M[o,k] * s[32c+k, col]
    M_lhsT = np.zeros((128, 128), np.float32)
    for cc in range(CH):
        M_lhsT[NS * cc:NS * (cc + 1), NS * cc:NS * (cc + 1)] = M.T
    # Final: out[t, 7c+j] = sum_{o} s7[32c+o, t] * F_rhs[32c+o, 7c+j]
    F_rhs = np.zeros((128, CH * NO), np.float32)
    for cc in range(CH):
        F_rhs[NS * cc:NS * (cc + 1), NO * cc:NO * (cc + 1)] = F.T
    # Per-partition bias vectors: col 0 = g (first iter), col 1 = c, col 2 = bact
    bias = np.zeros((128, 3), np.float32)
    bias[:, 0] = np.tile(g, CH)
    bias[:, 1] = np.tile(c, CH)
    bias[:, 2] = float(bact[0])
    identity = np.eye(128, dtype=np.float32)
    return G_lhsT, M_lhsT, F_rhs, bias, identity


def build_tile_kernel(ctx, tc, x, y, gw, mw, fw, idm, bvec, rows, nrep=1):
    """Emit the Tile program. x:[rows,18], y:[rows,7] DRAM APs; consts in DRAM.

    Two independent big-batch streams (slot 0/1) are emitted with their
    per-iteration stages interleaved so the in-order ScalarE stream alternates
    tanh(A), tanh(B) back-to-back while the PE runs the other stream's matmuls.

    nrep > 1 wraps the whole body in a hardware For_i loop that recomputes the
    same outputs nrep times — used only for wall-clock benchmarking."""
    nc = tc.nc
    full_tiles = rows // (CH * T_FULL)
    tail_rows = rows - full_tiles * CH * T_FULL
    tail_t = tail_rows // CH
    assert tail_t * CH == tail_rows

    consts = ctx.enter_context(tc.tile_pool(name="consts", bufs=1))
    # per-stream SBUF pools
    xsb_p = [ctx.enter_context(tc.tile_pool(name=f"xsb{s}", bufs=2)) for s in (0, 1)]
    xf_p = [ctx.enter_context(tc.tile_pool(name=f"xf{s}", bufs=2)) for s in (0, 1)]
    s_p = [ctx.enter_context(tc.tile_pool(name=f"sp{s}", bufs=2)) for s in (0, 1)]
    osb_p = [ctx.enter_context(tc.tile_pool(name=f"osb{s}", bufs=2)) for s in (0, 1)]
    # PSUM: 2 shared xt bufs (2 banks) + one 3-bank pre per stream = 8 banks
    xt_pool = ctx.enter_context(tc.tile_pool(name="xt", bufs=2, space="PSUM"))
    pre_p = [ctx.enter_context(tc.tile_pool(name=f"pre{s}", bufs=1, space="PSUM"))
             for s in (0, 1)]

    TANH = mybir.ActivationFunctionType.Tanh
    # float32r: same 4-byte storage, but the PE streams it at 1 cycle/col
    # (fp32 runs as 2 half-speed passes = 4 cycles/col). The BIR verifier
    # requires every producer feeding an f32r matmul to be typed f32r.
    F32R = mybir.dt.float32r
    x = x.bitcast(F32R)
    gw = gw.bitcast(F32R)
    mw = mw.bitcast(F32R)
    fw = fw.bitcast(F32R)
    idm = idm.bitcast(F32R)

    g_sb = consts.tile([CH * NF, 128], F32R, name="g_sb")
    nc.sync.dma_start(g_sb[:], gw)
    m_sb = consts.tile([128, 128], F32R, name="m_sb")
    nc.sync.dma_start(m_sb[:], mw)
    f_sb = consts.tile([128, CH * NO], F32R, name="f_sb")
    nc.sync.dma_start(f_sb[:], fw)
    id_sb = consts.tile([128, 128], F32R, name="id_sb")
    nc.sync.dma_start(id_sb[:], idm)
    b_sb = consts.tile([128, 3], F32, name="b_sb")
    nc.sync.dma_start(b_sb[:], bvec)

    CAP = T_FULL * BB_TILES  # tile capacity (columns) of the per-stream bufs

    def load_x(sl, bb):
        row0, ntile, t_last = bb
        widths = [T_FULL] * (ntile - 1) + [t_last]
        nrows = CH * sum(widths)
        dense = t_last == T_FULL
        x_sb = xsb_p[sl].tile([128, NF * CH * BB_TILES], F32R,
                              name=f"x_sb{sl}", tag="x_sb")
        if dense:
            srcx = x[row0:row0 + nrows, :].rearrange(
                "(k t c) f -> t k c f", k=ntile, t=T_FULL, c=CH)
            dstx = x_sb[:, 0:NF * CH * ntile].rearrange(
                "t (k c f) -> t k c f", k=ntile, c=CH, f=NF)
            nc.sync.dma_start(dstx, srcx)
        else:
            for ti, w in enumerate(widths):
                r0 = row0 + CH * sum(widths[:ti])
                srcx = x[r0:r0 + CH * w, :].rearrange("(t c) f -> t c f", t=w, c=CH)
                dstx = x_sb[0:w, NF * CH * ti:NF * CH * (ti + 1)].rearrange(
                    "t (c f) -> t c f", c=CH, f=NF)
                nc.sync.dma_start(dstx, srcx)
        return x_sb

    def transpose_x(sl, bb, x_sb):
        row0, ntile, t_last = bb
        widths = [T_FULL] * (ntile - 1) + [t_last]
        xfeat = xf_p[sl].tile([CH * NF, CAP], F32R, name=f"xfeat{sl}", tag="xfeat")
        for xg0 in range(0, ntile, 4):
            gcnt = min(4, ntile - xg0)
            xt = xt_pool.tile([CH * NF, 512], F32R, name="xt", tag="xt")
            for i in range(gcnt):
                ti = xg0 + i
                w = widths[ti]
                nc.tensor.transpose(
                    out=xt[0:CH * NF, 128 * i:128 * i + w],
                    in_=x_sb[0:w, NF * CH * ti:NF * CH * (ti + 1)],
                    identity=id_sb[0:w, 0:w],
                )
            if t_last == T_FULL:
                nc.vector.tensor_copy(
                    xfeat[0:CH * NF, 128 * xg0:128 * xg0 + 128 * gcnt],
                    xt[0:CH * NF, 0:128 * gcnt])
            else:
                for i in range(gcnt):
                    ti = xg0 + i
                    w = widths[ti]
                    nc.vector.tensor_copy(
                        xfeat[0:CH * NF, 128 * ti:128 * ti + w],
                        xt[0:CH * NF, 128 * i:128 * i + w])
        return xfeat

    def mm_sliced(pre, lhsT, rhs_tile, rhs_parts, width):
        for q0 in range(0, width, 512):
            w = min(512, width - q0)
            nc.tensor.matmul(
                pre[:, q0:q0 + w],
                lhsT=lhsT,
                rhs=rhs_tile[0:rhs_parts, q0:q0 + w],
                start=True, stop=True)

    def emit_group(bbs):
        """bbs: list of (slot, bb) with bb=(row0, ntile, t_last); stages are
        interleaved across slots so ScalarE alternates streams."""
        st = {}
        for sl, bb in bbs:
            st[sl] = {"bb": bb}
            st[sl]["ncols"] = T_FULL * (bb[1] - 1) + bb[2]
            st[sl]["x_sb"] = load_x(sl, bb)
        for sl, bb in bbs:
            st[sl]["xfeat"] = transpose_x(sl, bb, st[sl]["x_sb"])
        # iteration 1 (init fused with first tanh step)
        for sl, bb in bbs:
            pre = pre_p[sl].tile([128, CAP], F32, name=f"pre{sl}", tag="pre")
            st[sl]["pre"] = pre
            mm_sliced(pre, g_sb[:], st[sl]["xfeat"], CH * NF, st[sl]["ncols"])
        for sl, bb in bbs:
            s_t = s_p[sl].tile([128, CAP], F32R, name=f"s{sl}", tag="s")
            st[sl]["s"] = s_t
            nc.scalar.activation(s_t[:, 0:st[sl]["ncols"]],
                                 st[sl]["pre"][:, 0:st[sl]["ncols"]], TANH,
                                 bias=b_sb[:, 0:1], scale=1.0)
        # iterations 2..7
        for _ in range(6):
            for sl, bb in bbs:
                mm_sliced(st[sl]["pre"], m_sb[:], st[sl]["s"], 128,
                          st[sl]["ncols"])
            for sl, bb in bbs:
                nc.scalar.activation(st[sl]["s"][:, 0:st[sl]["ncols"]],
                                     st[sl]["pre"][:, 0:st[sl]["ncols"]], TANH,
                                     bias=b_sb[:, 1:2], scale=1.0)
        # final per-tile matmuls write into the (now free) head of pre
        for sl, bb in bbs:
            row0, ntile, t_last = bb
            widths = [T_FULL] * (ntile - 1) + [t_last]
            for ti, w in enumerate(widths):
                nc.tensor.matmul(
                    st[sl]["pre"][0:w, CH * NO * ti:CH * NO * (ti + 1)],
                    lhsT=st[sl]["s"][:, 128 * ti:128 * ti + w],
                    rhs=f_sb[:],
                    start=True, stop=True)
        for sl, bb in bbs:
            row0, ntile, t_last = bb
            widths = [T_FULL] * (ntile - 1) + [t_last]
            dense = t_last == T_FULL
            out_sb = osb_p[sl].tile([128, CH * NO * BB_TILES], F32,
                                    name=f"out_sb{sl}", tag="out_sb")
            if dense:
                nc.vector.tensor_scalar_add(
                    out_sb[:, 0:CH * NO * ntile],
                    st[sl]["pre"][:, 0:CH * NO * ntile], b_sb[:, 2:3])
            else:
                for ti, w in enumerate(widths):
                    nc.vector.tensor_scalar_add(
                        out_sb[0:w, CH * NO * ti:CH * NO * (ti + 1)],
                        st[sl]["pre"][0:w, CH * NO * ti:CH * NO * (ti + 1)],
                        b_sb[0:w, 2:3])
            nrows = CH * sum(widths)
            if dense:
                dsty = y[row0:row0 + nrows, :].rearrange(
                    "(k t c) j -> t k c j", k=ntile, t=T_FULL, c=CH)
                srcy = out_sb[:, 0:CH * NO * ntile].rearrange(
                    "t (k c j) -> t k c j", k=ntile, c=CH, j=NO)
                nc.sync.dma_start(dsty, srcy)
            else:
                for ti, w in enumerate(widths):
                    r0 = row0 + CH * sum(widths[:ti])
                    dsty = y[r0:r0 + CH * w, :].rearrange(
                        "(t c) j -> t c j", t=w, c=CH)
                    srcy = out_sb[0:w, CH * NO * ti:CH * NO * (ti + 1)].rearrange(
                        "t (c j) -> t c j", c=CH, j=NO)
                    nc.sync.dma_start(dsty, srcy)

    def emit_all():
        # build the BB descriptor list
        bbs = []
        n_full_bb = full_tiles // BB_TILES
        for b in range(n_full_bb):
            bbs.append((b * BB_TILES * CH * T_FULL, BB_TILES, T_FULL))
        leftover = full_tiles - n_full_bb * BB_TILES
        row0 = n_full_bb * BB_TILES * CH * T_FULL
        if tail_t > 0:
            bbs.append((row0, leftover + 1, tail_t))
        elif leftover:
            bbs.append((row0, leftover, T_FULL))
        # pair them across the two streams
        for i in range(0, len(bbs), 2):
            group = [(0, bbs[i])]
            if i + 1 < len(bbs):
                group.append((1, bbs[i + 1]))
            emit_group(group)

    if nrep == 1:
        emit_all()
    else:
        with tc.For_i(0, nrep, 1):
            emit_all()


_CACHED = {}
PROFILE = False  # set True (e.g. from test.py) to capture an NTFF trace
LAST_RESULTS = None  # BassKernelResults of the most recent kernel() call


def _build_program(rows, nrep=1):
    nc = bacc.Bacc("TRN2", target_bir_lowering=False, debug=False,
                   num_devices=N_CORES)
    x = nc.dram_tensor("x", [rows, NF], F32, kind="ExternalInput").ap()
    y = nc.dram_tensor("y", [rows, NO], F32, kind="ExternalOutput").ap()
    gw = nc.dram_tensor("gw", [CH * NF, 128], F32, kind="ExternalInput").ap()
    mw = nc.dram_tensor("mw", [128, 128], F32, kind="ExternalInput").ap()
    fw = nc.dram_tensor("fw", [128, CH * NO], F32, kind="ExternalInput").ap()
    idm = nc.dram_tensor("idm", [128, 128], F32, kind="ExternalInput").ap()
    bvec = nc.dram_tensor("bvec", [128, 3], F32, kind="ExternalInput").ap()
    with tile.TileContext(nc) as tc, ExitStack() as ctx:
        build_tile_kernel(ctx, tc, x, y, gw, mw, fw, idm, bvec, rows, nrep=nrep)
    nc.compile()
    return nc


def kernel(x, Wj, bj, Wm, bm, Wih_j, Whh_j, bih_j, bhh_j,
           Wih_m, Whh_m, bih_m, bhh_m, Wact, bact):
    x = np.ascontiguousarray(np.asarray(x, dtype=np.float32))
    assert x.shape == (B_TOTAL, NF), x.shape
    G_lhsT, M_lhsT, F_rhs, bias, identity = build_host_constants(
        np.asarray(Wj, np.float32), np.asarray(bj, np.float32),
        np.asarray(Wm, np.float32), np.asarray(bm, np.float32),
        np.asarray(Wih_j, np.float32), np.asarray(Whh_j, np.float32),
        np.asarray(bih_j, np.float32), np.asarray(bhh_j, np.float32),
        np.asarray(Wih_m, np.float32), np.asarray(Whh_m, np.float32),
        np.asarray(bih_m, np.float32), np.asarray(bhh_m, np.float32),
        np.asarray(Wact, np.float32), np.asarray(bact, np.float32))

    if "nc" not in _CACHED:
        _CACHED["nc"] = _build_program(R)
    nc = _CACHED["nc"]

    in_maps = []
    for i in range(N_CORES):
        in_maps.append({
            "x": x[i * R:(i + 1) * R],
            "gw": G_lhsT, "mw": M_lhsT, "fw": F_rhs,
            "idm": identity, "bvec": bias,
        })
    res = run_bass_kernel_spmd(nc, in_maps, list(range(N_CORES)), trace=PROFILE)
    global LAST_RESULTS
    LAST_RESULTS = res
    out = np.concatenate([res.results[i]["y"] for i in range(N_CORES)], axis=0)
    return out


# revision 22
# speedup vs baseline: 2.3464x; 1.4105x over previous
"""Trainium2 Bass kernel for the AggregPolicy GNN message-passing model.

Math: the reference network is, per batch row x (18 features):
    s0 = E @ x_feats + d          (state s = [h_j[0..6] (7*4), h_m (4)] = 32 dims)
    s  = tanh(M @ s + c)          x 7   (chain-graph message passing folded into
                                         one dense 32x32 matrix M)
    out = F @ s + bact            (7 outputs)
The first iteration is fused with the init affine: s1 = tanh(G @ x + g) with
G = M @ E, g = M @ d + c.  All matrices are precomputed on the host from the
tiny model weights.

Layout on chip (per NeuronCore, pure data parallel over 8 cores):
  - 4 batch "chunks" x 32 state dims are stacked on the 128 SBUF partitions
    (block-diagonal G/M/F), batch runs along the free dimension.
  - batch rows are interleaved mod 4 across chunks, so each SBUF partition of
    an input tile holds 4 *consecutive* DRAM rows (288 B contiguous reads) and
    each partition of an output tile holds 4 consecutive rows of y (112 B
    contiguous writes).
  - Per 128-column tile: PE transposes x [128,72] -> [72,128] (features to
    partitions), then one matmul per iteration per 512-col slice; ScalarE
    applies tanh+bias straight PSUM->SBUF; final per-tile matmul uses the
    state tile itself as the stationary operand to emit batch-major outputs.
"""

import os

os.environ.setdefault("MYCRO_LOCAL_CACHE", "1")

from contextlib import ExitStack

import numpy as np

import concourse.bacc as bacc
import concourse.tile as tile
from concourse import mybir
from concourse.bass_utils import run_bass_kernel_spmd

F32 = mybir.dt.float32
BF16 = mybir.dt.bfloat16
FP16 = mybir.dt.float16
# iteration/final matmuls in 16-bit (PE streams 1 col/cycle even cold, FWL
# weight loads); tanh still reads/writes via fp32 PSUM so only the matmul
# inputs are rounded. fp16 keeps 10 mantissa bits (values are all O(1), so
# no range risk) vs bf16's 7. K_SDT: fp16 | bf16 | f32r.
S_DT_KIND = os.environ.get("K_SDT", "fp16")
S_BF16 = S_DT_KIND in ("bf16", "fp16")  # 16-bit path enabled

N_CORES = 8
B_TOTAL = 2_000_000
R = B_TOTAL // N_CORES  # 250000 rows per core
NF = 18  # input features
NS = 32  # state dims
NO = 7  # outputs
CH = 4  # batch chunks stacked on partitions (4*32 = 128)
T_FULL = 128  # batch columns per tile (rows per tile = 4*T_FULL = 512)
BB_TILES = 12  # tiles per big-batch (ACT op free dim = 128*BB_TILES)

FULL_TILES = R // (CH * T_FULL)  # 488
TAIL_ROWS = R - FULL_TILES * CH * T_FULL  # 144
TAIL_T = TAIL_ROWS // CH  # 36
assert TAIL_T * CH == TAIL_ROWS


def build_host_constants(Wj, bj, Wm, bm, Wih_j, Whh_j, bih_j, bhh_j,
                         Wih_m, Whh_m, bih_m, bhh_m, Wact, bact):
    """Fold the model into (G_lhsT [72,128], M_lhsT [128,128], F_rhs [128,28],
    bias [128,3]) in the block-diagonal on-chip layouts."""
    H = 4
    M = np.zeros((NS, NS), np.float32)
    c = np.zeros((NS,), np.float32)
    for n in range(7):
        r = slice(4 * n, 4 * n + 4)
        if n == 0:
            M[r, 28:32] += Wih_j  # prev neighbor of node 0 is h_m
        else:
            M[r, 4 * (n - 1):4 * n] += Wih_j
        if n < 6:
            M[r, 4 * (n + 1):4 * (n + 2)] += Wih_j
        M[r, 4 * n:4 * n + 4] += Whh_j
        c[r] = bih_j + bhh_j
    M[28:32, 0:4] = Wih_m
    M[28:32, 28:32] = Whh_m
    c[28:32] = bih_m + bhh_m

    E = np.zeros((NS, NF), np.float32)
    d = np.zeros((NS,), np.float32)
    for n in range(7):
        for h in range(H):
            E[4 * n + h, 4 + n] = Wj[h, 0]
            E[4 * n + h, 11 + n] = Wj[h, 1]
        d[4 * n:4 * n + 4] = bj
    E[28:32, 0:4] = Wm
    d[28:32] = bm

    F = np.zeros((NO, NS), np.float32)
    for n in range(7):
        F[n, 4 * n:4 * n + 4] = Wact[0]

    G = (M @ E).astype(np.float32)
    g = (M @ d + c).astype(np.float32)

    # Block-diagonal device layouts.
    # G matmul: out[32c+o, col] = sum_f G[o,f] * xT[18c+f, col]
    G_lhsT = np.zeros((CH * NF, 128), np.float32)
    for cc in range(CH):
        G_lhsT[NF * cc:NF * (cc + 1), NS * cc:NS * (cc + 1)] = G.T
    # M matmul: out[32c+o, col] = sum_k M[o,k] * s[32c+k, col]
    M_lhsT = np.zeros((128, 128), np.float32)
    for cc in range(CH):
        M_lhsT[NS * cc:NS * (cc + 1), NS * cc:NS * (cc + 1)] = M.T
    # Final: out[t, 7c+j] = sum_{o} s7[32c+o, t] * F_rhs[32c+o, 7c+j]
    F_rhs = np.zeros((128, CH * NO), np.float32)
    for cc in range(CH):
        F_rhs[NS * cc:NS * (cc + 1), NO * cc:NO * (cc + 1)] = F.T
    # Per-partition bias vectors: col 0 = g (first iter), col 1 = c, col 2 = bact
    bias = np.zeros((128, 3), np.float32)
    bias[:, 0] = np.tile(g, CH)
    bias[:, 1] = np.tile(c, CH)
    bias[:, 2] = float(bact[0])
    identity = np.eye(128, dtype=np.float32)
    return G_lhsT, M_lhsT, F_rhs, bias, identity


def build_tile_kernel(ctx, tc, x, y, gw, mw, fw, idm, bvec, rows, nrep=1):
    """Emit the Tile program. x:[rows,18], y:[rows,7] DRAM APs; consts in DRAM.

    Two independent big-batch streams (slot 0/1) are emitted with their
    per-iteration stages interleaved so the in-order ScalarE stream alternates
    tanh(A), tanh(B) back-to-back while the PE runs the other stream's matmuls.

    nrep > 1 wraps the whole body in a hardware For_i loop that recomputes the
    same outputs nrep times — used only for wall-clock benchmarking."""
    nc = tc.nc
    full_tiles = rows // (CH * T_FULL)
    tail_rows = rows - full_tiles * CH * T_FULL
    tail_t = tail_rows // CH
    assert tail_t * CH == tail_rows

    consts = ctx.enter_context(tc.tile_pool(name="consts", bufs=1))
    # per-stream SBUF pools
    xsb_p = [ctx.enter_context(tc.tile_pool(name=f"xsb{s}", bufs=2)) for s in (0, 1)]
    xf_p = [ctx.enter_context(tc.tile_pool(name=f"xf{s}", bufs=2)) for s in (0, 1)]
    s_p = [ctx.enter_context(tc.tile_pool(name=f"sp{s}", bufs=2)) for s in (0, 1)]
    osb_p = [ctx.enter_context(tc.tile_pool(name=f"osb{s}", bufs=2)) for s in (0, 1)]
    # PSUM: 1 shared xt buf (1 bank) + 1 shared outps buf (1 bank)
    # + one 3-bank pre per stream = 8 banks
    xt_pool = ctx.enter_context(tc.tile_pool(name="xt", bufs=1, space="PSUM"))
    ops_pool = ctx.enter_context(tc.tile_pool(name="ops", bufs=1, space="PSUM"))
    pre_p = [ctx.enter_context(tc.tile_pool(name=f"pre{s}", bufs=1, space="PSUM"))
             for s in (0, 1)]

    TANH = mybir.ActivationFunctionType.Tanh
    # benchmark-only ablation knobs (default = full correct kernel)
    N_TANH_ITERS = int(os.environ.get("K_ITERS", "7"))
    SKIP_F = bool(os.environ.get("K_SKIP_F"))
    SKIP_TR = bool(os.environ.get("K_SKIP_TR"))
    SKIP_DMA_IN = bool(os.environ.get("K_SKIP_DMA_IN"))
    SKIP_DMA_OUT = bool(os.environ.get("K_SKIP_DMA_OUT"))
    # float32r: same 4-byte storage, but the PE streams it at 1 cycle/col
    # (fp32 runs as 2 half-speed passes = 4 cycles/col). The BIR verifier
    # requires every producer feeding an f32r matmul to be typed f32r.
    F32R = mybir.dt.float32r
    SDT = {"bf16": BF16, "fp16": FP16}.get(S_DT_KIND, F32R)
    x = x.bitcast(F32R)
    gw = gw.bitcast(F32R)
    idm = idm.bitcast(F32R)
    if not S_BF16:
        mw = mw.bitcast(F32R)
        fw = fw.bitcast(F32R)

    g_sb = consts.tile([CH * NF, 128], F32R, name="g_sb")
    nc.sync.dma_start(g_sb[:], gw)
    m_sb = consts.tile([128, 128], SDT, name="m_sb")
    nc.sync.dma_start(m_sb[:], mw)
    f_sb = consts.tile([128, CH * NO], SDT, name="f_sb")
    nc.sync.dma_start(f_sb[:], fw)
    id_sb = consts.tile([128, 128], F32R, name="id_sb")
    nc.sync.dma_start(id_sb[:], idm)
    b_sb = consts.tile([128, 3], F32, name="b_sb")
    nc.sync.dma_start(b_sb[:], bvec)

    CAP = T_FULL * BB_TILES  # tile capacity (columns) of the per-stream bufs

    def load_x(sl, bb):
        row0, ntile, t_last = bb
        widths = [T_FULL] * (ntile - 1) + [t_last]
        nrows = CH * sum(widths)
        dense = t_last == T_FULL
        x_sb = xsb_p[sl].tile([128, NF * CH * BB_TILES], F32R,
                              name=f"x_sb{sl}", tag="x_sb")
        if SKIP_DMA_IN:
            # bench-only: tiny write so Tile sees the tile as allocated
            nc.vector.memset(x_sb[:, 0:2], 0.5)
        elif dense:
            srcx = x[row0:row0 + nrows, :].rearrange(
                "(k t c) f -> t k c f", k=ntile, t=T_FULL, c=CH)
            dstx = x_sb[:, 0:NF * CH * ntile].rearrange(
                "t (k c f) -> t k c f", k=ntile, c=CH, f=NF)
            nc.sync.dma_start(dstx, srcx)
        else:
            for ti, w in enumerate(widths):
                r0 = row0 + CH * sum(widths[:ti])
                srcx = x[r0:r0 + CH * w, :].rearrange("(t c) f -> t c f", t=w, c=CH)
                dstx = x_sb[0:w, NF * CH * ti:NF * CH * (ti + 1)].rearrange(
                    "t (c f) -> t c f", c=CH, f=NF)
                nc.sync.dma_start(dstx, srcx)
        return x_sb

    def transpose_x(sl, bb, x_sb):
        row0, ntile, t_last = bb
        widths = [T_FULL] * (ntile - 1) + [t_last]
        xfeat = xf_p[sl].tile([CH * NF, CAP], F32R, name=f"xfeat{sl}", tag="xfeat")
        if SKIP_TR:
            return xfeat
        for xg0 in range(0, ntile, 4):
            gcnt = min(4, ntile - xg0)
            xt = xt_pool.tile([CH * NF, 512], F32R, name="xt", tag="xt")
            for i in range(gcnt):
                ti = xg0 + i
                w = widths[ti]
                nc.tensor.transpose(
                    out=xt[0:CH * NF, 128 * i:128 * i + w],
                    in_=x_sb[0:w, NF * CH * ti:NF * CH * (ti + 1)],
                    identity=id_sb[0:w, 0:w],
                )
            if t_last == T_FULL:
                nc.vector.tensor_copy(
                    xfeat[0:CH * NF, 128 * xg0:128 * xg0 + 128 * gcnt],
                    xt[0:CH * NF, 0:128 * gcnt])
            else:
                for i in range(gcnt):
                    ti = xg0 + i
                    w = widths[ti]
                    nc.vector.tensor_copy(
                        xfeat[0:CH * NF, 128 * ti:128 * ti + w],
                        xt[0:CH * NF, 128 * i:128 * i + w])
        return xfeat

    def mm_sliced(pre, lhsT, rhs_tile, rhs_parts, width):
        for q0 in range(0, width, 512):
            w = min(512, width - q0)
            nc.tensor.matmul(
                pre[:, q0:q0 + w],
                lhsT=lhsT,
                rhs=rhs_tile[0:rhs_parts, q0:q0 + w],
                start=True, stop=True)

    def emit_group(bbs):
        """bbs: list of (slot, bb) with bb=(row0, ntile, t_last); stages are
        interleaved across slots so ScalarE alternates streams."""
        st = {}
        for sl, bb in bbs:
            st[sl] = {"bb": bb}
            st[sl]["ncols"] = T_FULL * (bb[1] - 1) + bb[2]
            st[sl]["x_sb"] = load_x(sl, bb)
        for sl, bb in bbs:
            st[sl]["xfeat"] = transpose_x(sl, bb, st[sl]["x_sb"])
        # iteration 1 (init fused with first tanh step)
        for sl, bb in bbs:
            pre = pre_p[sl].tile([128, CAP], F32, name=f"pre{sl}", tag="pre")
            st[sl]["pre"] = pre
            mm_sliced(pre, g_sb[:], st[sl]["xfeat"], CH * NF, st[sl]["ncols"])
        for sl, bb in bbs:
            s_t = s_p[sl].tile([128, CAP], SDT, name=f"s{sl}", tag="s")
            st[sl]["s"] = s_t
            nc.scalar.activation(s_t[:, 0:st[sl]["ncols"]],
                                 st[sl]["pre"][:, 0:st[sl]["ncols"]], TANH,
                                 bias=b_sb[:, 0:1], scale=1.0)
        # iterations 2..7
        for _ in range(N_TANH_ITERS - 1):
            for sl, bb in bbs:
                mm_sliced(st[sl]["pre"], m_sb[:], st[sl]["s"], 128,
                          st[sl]["ncols"])
            for sl, bb in bbs:
                nc.scalar.activation(st[sl]["s"][:, 0:st[sl]["ncols"]],
                                     st[sl]["pre"][:, 0:st[sl]["ncols"]], TANH,
                                     bias=b_sb[:, 1:2], scale=1.0)
        # final per-tile matmuls go to a separate PSUM bank so they stay off
        # the pre-tile WAR chain (next BB's G-matmul only waits on tanh7).
        for sl, bb in bbs:
            if SKIP_F:
                continue
            row0, ntile, t_last = bb
            widths = [T_FULL] * (ntile - 1) + [t_last]
            outps = ops_pool.tile([128, CH * NO * BB_TILES], F32,
                                  name="outps", tag="outps")
            st[sl]["outps"] = outps
            for ti, w in enumerate(widths):
                nc.tensor.matmul(
                    outps[0:w, CH * NO * ti:CH * NO * (ti + 1)],
                    lhsT=st[sl]["s"][:, 128 * ti:128 * ti + w],
                    rhs=f_sb[:],
                    start=True, stop=True)
        for sl, bb in bbs:
            if SKIP_F:
                continue
            row0, ntile, t_last = bb
            widths = [T_FULL] * (ntile - 1) + [t_last]
            dense = t_last == T_FULL
            out_sb = osb_p[sl].tile([128, CH * NO * BB_TILES], F32,
                                    name=f"out_sb{sl}", tag="out_sb")
            outps = st[sl]["outps"]
            if dense:
                nc.vector.tensor_scalar_add(
                    out_sb[:, 0:CH * NO * ntile],
                    outps[:, 0:CH * NO * ntile], b_sb[:, 2:3])
            else:
                for ti, w in enumerate(widths):
                    nc.vector.tensor_scalar_add(
                        out_sb[0:w, CH * NO * ti:CH * NO * (ti + 1)],
                        outps[0:w, CH * NO * ti:CH * NO * (ti + 1)],
                        b_sb[0:w, 2:3])
            nrows = CH * sum(widths)
            if SKIP_DMA_OUT:
                continue
            if dense:
                dsty = y[row0:row0 + nrows, :].rearrange(
                    "(k t c) j -> t k c j", k=ntile, t=T_FULL, c=CH)
                srcy = out_sb[:, 0:CH * NO * ntile].rearrange(
                    "t (k c j) -> t k c j", k=ntile, c=CH, j=NO)
                nc.sync.dma_start(dsty, srcy)
            else:
                for ti, w in enumerate(widths):
                    r0 = row0 + CH * sum(widths[:ti])
                    dsty = y[r0:r0 + CH * w, :].rearrange(
                        "(t c) j -> t c j", t=w, c=CH)
                    srcy = out_sb[0:w, CH * NO * ti:CH * NO * (ti + 1)].rearrange(
                        "t (c j) -> t c j", c=CH, j=NO)
                    nc.sync.dma_start(dsty, srcy)

    def emit_all():
        # build the BB descriptor list
        bbs = []
        n_full_bb = full_tiles // BB_TILES
        for b in range(n_full_bb):
            bbs.append((b * BB_TILES * CH * T_FULL, BB_TILES, T_FULL))
        leftover = full_tiles - n_full_bb * BB_TILES
        row0 = n_full_bb * BB_TILES * CH * T_FULL
        if tail_t > 0:
            bbs.append((row0, leftover + 1, tail_t))
        elif leftover:
            bbs.append((row0, leftover, T_FULL))
        # pair them across the two streams
        for i in range(0, len(bbs), 2):
            group = [(0, bbs[i])]
            if i + 1 < len(bbs):
                group.append((1, bbs[i + 1]))
            emit_group(group)

    if nrep == 1:
        emit_all()
    else:
        with tc.For_i(0, nrep, 1):
            emit_all()


_CACHED = {}
PROFILE = False  # set True (e.g. from test.py) to capture an NTFF trace
LAST_RESULTS = None  # BassKernelResults of the most recent kernel() call


def _build_program(rows, nrep=1):
    nc = bacc.Bacc("TRN2", target_bir_lowering=False, debug=False,
                   num_devices=N_CORES)
    x = nc.dram_tensor("x", [rows, NF], F32, kind="ExternalInput").ap()
    y = nc.dram_tensor("y", [rows, NO], F32, kind="ExternalOutput").ap()
    gw = nc.dram_tensor("gw", [CH * NF, 128], F32, kind="ExternalInput").ap()
    wdt = {"bf16": BF16, "fp16": FP16}.get(S_DT_KIND, F32)
    mw = nc.dram_tensor("mw", [128, 128], wdt, kind="ExternalInput").ap()
    fw = nc.dram_tensor("fw", [128, CH * NO], wdt, kind="ExternalInput").ap()
    idm = nc.dram_tensor("idm", [128, 128], F32, kind="ExternalInput").ap()
    bvec = nc.dram_tensor("bvec", [128, 3], F32, kind="ExternalInput").ap()
    with tile.TileContext(nc) as tc, ExitStack() as ctx:
        build_tile_kernel(ctx, tc, x, y, gw, mw, fw, idm, bvec, rows, nrep=nrep)
    nc.compile()
    return nc


def kernel(x, Wj, bj, Wm, bm, Wih_j, Whh_j, bih_j, bhh_j,
           Wih_m, Whh_m, bih_m, bhh_m, Wact, bact):
    x = np.ascontiguousarray(np.asarray(x, dtype=np.float32))
    assert x.shape == (B_TOTAL, NF), x.shape
    G_lhsT, M_lhsT, F_rhs, bias, identity = build_host_constants(
        np.asarray(Wj, np.float32), np.asarray(bj, np.float32),
        np.asarray(Wm, np.float32), np.asarray(bm, np.float32),
        np.asarray(Wih_j, np.float32), np.asarray(Whh_j, np.float32),
        np.asarray(bih_j, np.float32), np.asarray(bhh_j, np.float32),
        np.asarray(Wih_m, np.float32), np.asarray(Whh_m, np.float32),
        np.asarray(bih_m, np.float32), np.asarray(bhh_m, np.float32),
        np.asarray(Wact, np.float32), np.asarray(bact, np.float32))

    if "nc" not in _CACHED:
        _CACHED["nc"] = _build_program(R)
    nc = _CACHED["nc"]

    if S_DT_KIND == "bf16":
        import ml_dtypes
        M_lhsT = M_lhsT.astype(ml_dtypes.bfloat16)
        F_rhs = F_rhs.astype(ml_dtypes.bfloat16)
    elif S_DT_KIND == "fp16":
        M_lhsT = M_lhsT.astype(np.float16)
        F_rhs = F_rhs.astype(np.float16)
    in_maps = []
    for i in range(N_CORES):
        in_maps.append({
            "x": x[i * R:(i + 1) * R],
            "gw": G_lhsT, "mw": M_lhsT, "fw": F_rhs,
            "idm": identity, "bvec": bias,
        })
    res = run_bass_kernel_spmd(nc, in_maps, list(range(N_CORES)), trace=PROFILE)
    global LAST_RESULTS
    LAST_RESULTS = res
    out = np.concatenate([res.results[i]["y"] for i in range(N_CORES)], axis=0)
    return out
